# revision 20
# baseline (speedup 1.0000x reference)
"""Trainium2 Bass kernel for the tanh-attention module (nn_Attention_1580547969144).

Computation (per batch row b):
    enc_contrib = enc_states @ W_enc.T + b_enc            # [S, A]
    dec_contrib = dec_state @ W_dec.T                     # [A]
    pre = tanh(enc_contrib + dec_contrib)                 # [S, A]
    energy = pre @ w_attn                                 # [S]
    alpha = softmax(energy) * mask; alpha /= sum(alpha)   # [S]
    context = alpha @ enc_states                          # [E]

Sharding: data-parallel over batch, 4 rows per core across 8 NeuronCores.
On-chip strategy (per core):
  - All matmuls run in bf16 with fp32 PSUM accumulation.
  - enc is cast fp32->bf16 during the SWDGE DMA load, round-tripped through a
    DRAM bf16 scratch, and re-loaded with the HWDGE xbar DMA transpose to get
    the [E, S] layout the enc @ W_enc.T contraction needs.  Same staging is
    used once for W_enc.T / W_dec.T.
  - The big [A, S] projection is computed transposed (A on partitions) so that
    the energy reduction over A is a K-partition matmul with w_attn.
  - tanh runs on ScalarE straight out of PSUM with a per-partition fused bias
    (dec_contrib.T + b_enc), writing bf16 to SBUF.
  - Softmax over S for all 4 rows at once on partitions 0..3; the exp*mask and
    its row sum are fused into one tensor_tensor_reduce.
  - context uses the natural-layout bf16 enc tiles kept from the load, with
    alpha.T produced by tiny K=4 identity matmuls.
"""

import sys

if "/opt/trn_rl_repo" not in sys.path:
    sys.path.insert(0, "/opt/trn_rl_repo")

import numpy as np

import concourse.bass as bass  # noqa: F401  (namespace import keeps bass alive)
import concourse.mybir as mybir
import concourse.tile as tile
from concourse import bacc, bass_utils
from concourse.bass import ts
from concourse.masks import make_identity

B, S, E, A, D = 32, 1024, 1024, 1024, 1024
NCORES = 8
BL = B // NCORES  # 4 batch rows per core
P = 128
SC, EC, AC, DC = S // P, E // P, A // P, D // P
F32 = mybir.dt.float32
BF16 = mybir.dt.bfloat16
AF = mybir.ActivationFunctionType
ALU = mybir.AluOpType
AX = mybir.AxisListType

_compiled = None


def _build_kernel():
    nc = bacc.Bacc(
        "TRN2",
        target_bir_lowering=False,
        debug=False,
        enable_asserts=False,
        num_devices=NCORES,
    )

    dec_ap = nc.dram_tensor("dec_state", [BL, D], F32, kind="ExternalInput").ap()
    enc_ap = nc.dram_tensor("enc_states", [BL, S, E], F32, kind="ExternalInput").ap()
    mask_ap = nc.dram_tensor("mask", [BL, S], F32, kind="ExternalInput").ap()
    wenc_ap = nc.dram_tensor("W_enc", [A, E], F32, kind="ExternalInput").ap()
    benc_ap = nc.dram_tensor("b_enc", [A], F32, kind="ExternalInput").ap()
    wdec_ap = nc.dram_tensor("W_dec", [A, D], F32, kind="ExternalInput").ap()
    wattn_ap = nc.dram_tensor("w_attn", [A], F32, kind="ExternalInput").ap()
    ctx_out = nc.dram_tensor("context", [BL, E], F32, kind="ExternalOutput").ap()
    alpha_out = nc.dram_tensor("alpha", [BL, S], F32, kind="ExternalOutput").ap()

    with tile.TileContext(nc) as tc:
        with (
            tc.tile_pool(name="persist", bufs=1) as persist,
            tc.tile_pool(name="encnat", bufs=BL) as p_encnat,
            tc.tile_pool(name="encT", bufs=2) as p_encT,
            tc.tile_pool(name="pre", bufs=2) as p_pre,
            tc.tile_pool(name="ppmain", bufs=2, space="PSUM") as pp_main,
            tc.tile_pool(name="ppsmall", bufs=2, space="PSUM") as pp_small,
        ):
            # ---------------- persistent small tiles ----------------
            ident16 = persist.tile([P, P], BF16, tag="ident16")
            make_identity(nc, ident16)
            ones32 = persist.tile([1, 1], F32, tag="ones32")
            nc.vector.memset(ones32, 1.0)
            # diag4[p, b, m] = (b == m), replicated on every partition
            diag4 = persist.tile([P, BL, BL], BF16, tag="diag4")
            nc.vector.memset(diag4, 0.0)
            for j in range(BL):
                nc.vector.memset(diag4[:, j, j : j + 1], 1.0)

            w_encT = persist.tile([P, EC, A], BF16, tag="w_encT")  # [e_p, ec, a]
            w_decT = persist.tile([P, DC, A], BF16, tag="w_decT")  # [d_p, dc, a]
            b_encT = persist.tile([P, AC], F32, tag="b_encT")  # [a_p, ac]
            w_attnT = persist.tile([P, AC], BF16, tag="w_attnT")  # [a_p, ac]
            # one-hot expansions: [.., b, m] = value * (b == m); column m of the
            # [K, 4] lhsT slice [:, c, b, :] carries the vector only for m == b,
            # so all 4 batch rows can accumulate into one [4, N] PSUM tile
            w_attn_oh = persist.tile([P, AC, BL, BL], BF16, tag="w_attn_oh")
            alphaT_oh = persist.tile([P, SC, BL, BL], BF16, tag="alphaT_oh")
            bias_sb = persist.tile([P, AC, BL], F32, tag="bias_sb")
            dec16 = persist.tile([BL, D], BF16, tag="dec16")
            decT = persist.tile([P, DC, BL], BF16, tag="decT")  # [d_p, dc, b]
            mask_sb = persist.tile([BL, S], F32, tag="mask_sb")
            b1 = persist.tile([1, A], F32, tag="b1")
            w1 = persist.tile([1, A], F32, tag="w1")
            w1b = persist.tile([1, A], BF16, tag="w1b")

            # softmax / output tiles; stats columns: 0=max, 1=-max, 2=sum, 3=1/sum
            stats = persist.tile([BL, 4], F32, tag="stats")
            masked = persist.tile([BL, S], F32, tag="masked")
            alpha16 = persist.tile([BL, S], BF16, tag="alpha16")
            alphaT = persist.tile([P, SC, BL], BF16, tag="alphaT")  # [s_p, sc, b]
            ctx_sb = persist.tile([BL, E], F32, tag="ctx_sb")

            # ---------------- small input loads ----------------
            nc.sync.dma_start(mask_sb, mask_ap)
            nc.sync.dma_start(b1, benc_ap[None, :])
            nc.sync.dma_start(w1, wattn_ap[None, :])
            nc.gpsimd.dma_start(dec16, dec_ap)  # fp32 -> bf16 cast in DMA
            nc.vector.tensor_copy(out=w1b, in_=w1)

            # distribute the 1-D vectors across partitions ([1,128] -> [128,1])
            # via tiny K=1 / K=4 matmuls against an identity column
            for c in range(AC):
                pst = pp_small.tile([P, BL], F32, tag="pssmall")
                nc.tensor.matmul(
                    pst[:, 0:1], b1[0:1, ts(c, P)], ones32[0:1, 0:1],
                    start=True, stop=True,
                )
                nc.vector.tensor_copy(out=b_encT[:, c : c + 1], in_=pst[:, 0:1])
            for c in range(AC):
                pst = pp_small.tile([P, BL], F32, tag="pssmall")
                nc.tensor.matmul(
                    pst[:, 0:1], w1b[0:1, ts(c, P)], ident16[0:1, 0:1],
                    start=True, stop=True,
                )
                nc.vector.tensor_copy(out=w_attnT[:, c : c + 1], in_=pst[:, 0:1])
            nc.vector.tensor_tensor(
                w_attn_oh,
                w_attnT[:, :, None, None].to_broadcast([P, AC, BL, BL]),
                diag4[:, None, :, :].to_broadcast([P, AC, BL, BL]),
                ALU.mult,
            )
            for c in range(DC):
                pst = pp_small.tile([P, BL], F32, tag="pssmall")
                nc.tensor.matmul(
                    pst, dec16[:, ts(c, P)], ident16[0:BL, 0:BL],
                    start=True, stop=True,
                )
                nc.vector.tensor_copy(out=decT[:, c, :], in_=pst)

            # -------- weight staging: cast-load bf16, SBUF->SBUF xbar blocks --
            # staging tiles borrow slots from the encnat pool (same shape)
            for w_ap, wT in ((wenc_ap, w_encT), (wdec_ap, w_decT)):
                wst = p_encnat.tile([P, AC, E], BF16, tag="encnat")
                for cc in range(AC):
                    nc.gpsimd.dma_start(wst[:, cc, :], w_ap[ts(cc, P), :])
                for cc in range(AC):
                    for ec_ in range(EC):
                        nc.sync.dma_start(
                            wT[:, ec_, ts(cc, P)],
                            wst[:, cc, ts(ec_, P)],
                            transpose=True,
                        )

            # ---------------- dec contribution + fused bias ----------------
            # bias_sb[a_p, ac, b] = (W_dec @ dec.T)[a, b] + b_enc[a]
            for ac_ in range(AC):
                psd = pp_small.tile([P, BL], F32, tag="pssmall")
                for dc_ in range(DC):
                    nc.tensor.matmul(
                        psd,
                        w_decT[:, dc_, ts(ac_, P)],
                        decT[:, dc_, :],
                        start=(dc_ == 0),
                        stop=(dc_ == DC - 1),
                    )
                nc.vector.tensor_scalar_add(
                    bias_sb[:, ac_, :], psd, b_encT[:, ac_ : ac_ + 1]
                )

            # ---------------- per-batch pipeline ----------------
            # all 4 rows' energies accumulate into one [4, S] PSUM tile via the
            # one-hot lhsT columns
            ps_energy = pp_small.tile([BL, S], F32, tag="pssmall")
            encnat_tiles = []
            for b in range(BL):
                encnat = p_encnat.tile([P, SC, E], BF16, tag="encnat")
                encnat_tiles.append(encnat)
                for sc_ in range(SC):
                    nc.gpsimd.dma_start(encnat[:, sc_, :], enc_ap[b][ts(sc_, P), :])
                encT = p_encT.tile([P, EC, S], BF16, tag="encT")
                for sc_ in range(SC):
                    for ec_ in range(EC):
                        nc.sync.dma_start(
                            encT[:, ec_, ts(sc_, P)],
                            encnat[:, sc_, ts(ec_, P)],
                            transpose=True,
                        )

                pre = p_pre.tile([P, AC, S], BF16, tag="pre")
                for ac_ in range(AC):
                    ps = pp_main.tile([P, S], F32, tag="psmain")
                    for ec_ in range(EC):
                        for nh in range(2):
                            nc.tensor.matmul(
                                ps[:, ts(nh, 512)],
                                w_encT[:, ec_, ts(ac_, P)],
                                encT[:, ec_, ts(nh, 512)],
                                start=(ec_ == 0),
                                stop=(ec_ == EC - 1),
                            )
                    # pre = tanh(enc_contrib.T + dec_contrib.T + b_enc), bf16 out
                    nc.scalar.activation(
                        pre[:, ac_, :], ps, AF.Tanh,
                        bias=bias_sb[:, ac_, b : b + 1], scale=1.0,
                    )
                    for nh in range(2):
                        nc.tensor.matmul(
                            ps_energy[0:BL, ts(nh, 512)],
                            w_attn_oh[:, ac_, b, :],
                            pre[:, ac_, ts(nh, 512)],
                            start=(b == 0 and ac_ == 0),
                            stop=(b == BL - 1 and ac_ == AC - 1),
                        )

            # ---------------- masked softmax over S (all 4 rows) -------------
            # max + exp read the energies straight out of PSUM
            nc.vector.tensor_reduce(stats[:, 0:1], ps_energy, axis=AX.X, op=ALU.max)
            nc.vector.tensor_scalar_mul(stats[:, 1:2], stats[:, 0:1], -1.0)
            nc.scalar.activation(masked, ps_energy, AF.Exp, bias=stats[:, 1:2], scale=1.0)
            # masked = exp * mask; stats[:,2] = rowsum(masked)
            # (tensor_tensor_reduce would fuse these, but the custom DVE op
            # crashes at execute through this runtime path — keep it unfused)
            nc.vector.tensor_tensor(masked, masked, mask_sb, ALU.mult)
            nc.vector.tensor_reduce(stats[:, 2:3], masked, axis=AX.X, op=ALU.add)
            nc.vector.reciprocal(stats[:, 3:4], stats[:, 2:3])
            nc.vector.tensor_scalar_mul(masked, masked, stats[:, 3:4])
            nc.sync.dma_start(alpha_out, masked)
            nc.vector.tensor_copy(out=alpha16, in_=masked)

            # alpha.T: [4, S] -> [s_p, sc, 4] with K=4 identity matmuls
            for sc_ in range(SC):
                pst = pp_small.tile([P, BL], F32, tag="pssmall")
                nc.tensor.matmul(
                    pst, alpha16[:, ts(sc_, P)], ident16[0:BL, 0:BL],
                    start=True, stop=True,
                )
                nc.vector.tensor_copy(out=alphaT[:, sc_, :], in_=pst)
            nc.vector.tensor_tensor(
                alphaT_oh,
                alphaT[:, :, :, None].to_broadcast([P, SC, BL, BL]),
                diag4[:, None, :, :].to_broadcast([P, SC, BL, BL]),
                ALU.mult,
            )

            # ---------------- context = alpha @ enc ----------------
            psc = pp_small.tile([BL, E], F32, tag="pssmall")
            for b in range(BL):
                for sc_ in range(SC):
                    for nh in range(2):
                        nc.tensor.matmul(
                            psc[0:BL, ts(nh, 512)],
                            alphaT_oh[:, sc_, b, :],
                            encnat_tiles[b][:, sc_, ts(nh, 512)],
                            start=(b == 0 and sc_ == 0),
                            stop=(b == BL - 1 and sc_ == SC - 1),
                        )
            nc.vector.tensor_copy(out=ctx_sb, in_=psc)
            nc.sync.dma_start(ctx_out, ctx_sb)

    nc.compile()
    return nc


def _get_compiled():
    global _compiled
    if _compiled is None:
        _compiled = _build_kernel()
    return _compiled


def kernel(dec_state, enc_states, mask, W_enc, b_enc, W_dec, w_attn):
    nc = _get_compiled()

    shared = {
        "W_enc": np.ascontiguousarray(W_enc, dtype=np.float32),
        "b_enc": np.ascontiguousarray(b_enc, dtype=np.float32),
        "W_dec": np.ascontiguousarray(W_dec, dtype=np.float32),
        "w_attn": np.ascontiguousarray(w_attn, dtype=np.float32),
    }
    in_maps = []
    for c in range(NCORES):
        sl = slice(c * BL, (c + 1) * BL)
        in_maps.append(
            {
                "dec_state": np.ascontiguousarray(dec_state[sl], dtype=np.float32),
                "enc_states": np.ascontiguousarray(enc_states[sl], dtype=np.float32),
                "mask": np.ascontiguousarray(mask[sl], dtype=np.float32),
                **shared,
            }
        )

    res = bass_utils.run_bass_kernel_spmd(nc, in_maps, core_ids=list(range(NCORES)))
    kernel.last_results = res
    context = np.concatenate([r["context"] for r in res.results], axis=0)
    alpha = np.concatenate([r["alpha"] for r in res.results], axis=0)
    return (context, alpha)


# revision 23
# speedup vs baseline: 2.5647x; 2.5647x over previous
"""Trainium2 Bass kernel for the tanh-attention module (nn_Attention_1580547969144).

Computation (per batch row b):
    enc_contrib = enc_states @ W_enc.T + b_enc            # [S, A]
    dec_contrib = dec_state @ W_dec.T                     # [A]
    pre = tanh(enc_contrib + dec_contrib)                 # [S, A]
    energy = pre @ w_attn                                 # [S]
    alpha = softmax(energy) * mask; alpha /= sum(alpha)   # [S]
    context = alpha @ enc_states                          # [E]

Sharding: data-parallel over batch, 4 rows per core across 8 NeuronCores.
On-chip strategy (per core):
  - All matmuls run in bf16 with fp32 PSUM accumulation.
  - enc is cast fp32->bf16 during the SWDGE DMA load, round-tripped through a
    DRAM bf16 scratch, and re-loaded with the HWDGE xbar DMA transpose to get
    the [E, S] layout the enc @ W_enc.T contraction needs.  Same staging is
    used once for W_enc.T / W_dec.T.
  - The big [A, S] projection is computed transposed (A on partitions) so that
    the energy reduction over A is a K-partition matmul with w_attn.
  - tanh runs on ScalarE straight out of PSUM with a per-partition fused bias
    (dec_contrib.T + b_enc), writing bf16 to SBUF.
  - Softmax over S for all 4 rows at once on partitions 0..3; the exp*mask and
    its row sum are fused into one tensor_tensor_reduce.
  - context uses the natural-layout bf16 enc tiles kept from the load, with
    alpha.T produced by tiny K=4 identity matmuls.
"""

import sys

if "/opt/trn_rl_repo" not in sys.path:
    sys.path.insert(0, "/opt/trn_rl_repo")

import numpy as np

import concourse.bass as bass  # noqa: F401  (namespace import keeps bass alive)
import concourse.mybir as mybir
import concourse.tile as tile
from concourse import bacc, bass_utils
from concourse.bass import ts
from concourse.masks import make_identity

B, S, E, A, D = 32, 1024, 1024, 1024, 1024
NCORES = 8
BL = B // NCORES  # 4 batch rows per core
P = 128
SC, EC, AC, DC = S // P, E // P, A // P, D // P
F32 = mybir.dt.float32
BF16 = mybir.dt.bfloat16
AF = mybir.ActivationFunctionType
ALU = mybir.AluOpType
AX = mybir.AxisListType

_compiled = None


def _build_kernel():
    nc = bacc.Bacc(
        "TRN2",
        target_bir_lowering=False,
        debug=False,
        enable_asserts=False,
        num_devices=NCORES,
    )

    dec_ap = nc.dram_tensor("dec_state", [BL, D], F32, kind="ExternalInput").ap()
    enc_ap = nc.dram_tensor("enc_states", [BL, S, E], F32, kind="ExternalInput").ap()
    mask_ap = nc.dram_tensor("mask", [BL, S], F32, kind="ExternalInput").ap()
    wenc_ap = nc.dram_tensor("W_enc", [A, E], F32, kind="ExternalInput").ap()
    benc_ap = nc.dram_tensor("b_enc", [A], F32, kind="ExternalInput").ap()
    wdec_ap = nc.dram_tensor("W_dec", [A, D], F32, kind="ExternalInput").ap()
    wattn_ap = nc.dram_tensor("w_attn", [A], F32, kind="ExternalInput").ap()
    ctx_out = nc.dram_tensor("context", [BL, E], F32, kind="ExternalOutput").ap()
    alpha_out = nc.dram_tensor("alpha", [BL, S], F32, kind="ExternalOutput").ap()

    with tile.TileContext(nc) as tc:
        with (
            tc.tile_pool(name="persist", bufs=1) as persist,
            tc.tile_pool(name="encnat", bufs=BL) as p_encnat,
            tc.tile_pool(name="encT", bufs=2) as p_encT,
            tc.tile_pool(name="pre", bufs=2) as p_pre,
            tc.tile_pool(name="dram", bufs=3, space="DRAM") as p_dram,
            tc.tile_pool(name="ppmain", bufs=2, space="PSUM") as pp_main,
            tc.tile_pool(name="ppsmall", bufs=2, space="PSUM") as pp_small,
        ):
            # ---------------- persistent small tiles ----------------
            ident16 = persist.tile([P, P], BF16, tag="ident16")
            make_identity(nc, ident16)
            ones32 = persist.tile([1, 1], F32, tag="ones32")
            nc.vector.memset(ones32, 1.0)
            # diag4[p, b, m] = (b == m), replicated on every partition
            diag4 = persist.tile([P, BL, BL], BF16, tag="diag4")
            nc.vector.memset(diag4, 0.0)
            for j in range(BL):
                nc.vector.memset(diag4[:, j, j : j + 1], 1.0)

            w_encT = persist.tile([P, EC, A], BF16, tag="w_encT")  # [e_p, ec, a]
            w_decT = persist.tile([P, DC, A], BF16, tag="w_decT")  # [d_p, dc, a]
            b_encT = persist.tile([P, AC], F32, tag="b_encT")  # [a_p, ac]
            w_attnT = persist.tile([P, AC], BF16, tag="w_attnT")  # [a_p, ac]
            # one-hot expansions: [.., b, m] = value * (b == m); column m of the
            # [K, 4] lhsT slice [:, c, b, :] carries the vector only for m == b,
            # so all 4 batch rows can accumulate into one [4, N] PSUM tile
            w_attn_oh = persist.tile([P, AC, BL, BL], BF16, tag="w_attn_oh")
            alphaT_oh = persist.tile([P, SC, BL, BL], BF16, tag="alphaT_oh")
            bias_sb = persist.tile([P, AC, BL], F32, tag="bias_sb")
            dec16 = persist.tile([BL, D], BF16, tag="dec16")
            decT = persist.tile([P, DC, BL], BF16, tag="decT")  # [d_p, dc, b]
            mask_sb = persist.tile([BL, S], F32, tag="mask_sb")
            b1 = persist.tile([1, A], F32, tag="b1")
            w1 = persist.tile([1, A], F32, tag="w1")
            w1b = persist.tile([1, A], BF16, tag="w1b")

            # softmax / output tiles; stats columns: 0=max, 1=-max, 2=sum, 3=1/sum
            stats = persist.tile([BL, 4], F32, tag="stats")
            masked = persist.tile([BL, S], F32, tag="masked")
            alpha16 = persist.tile([BL, S], BF16, tag="alpha16")
            alphaT = persist.tile([P, SC, BL], BF16, tag="alphaT")  # [s_p, sc, b]
            ctx_sb = persist.tile([BL, E], F32, tag="ctx_sb")

            # ---------------- small input loads ----------------
            nc.sync.dma_start(mask_sb, mask_ap)
            nc.sync.dma_start(b1, benc_ap[None, :])
            nc.sync.dma_start(w1, wattn_ap[None, :])
            nc.gpsimd.dma_start(dec16, dec_ap)  # fp32 -> bf16 cast in DMA
            nc.vector.tensor_copy(out=w1b, in_=w1)

            # distribute the 1-D vectors across partitions ([1,128] -> [128,1])
            # via tiny K=1 / K=4 matmuls against an identity column
            for c in range(AC):
                pst = pp_small.tile([P, BL], F32, tag="pssmall")
                nc.tensor.matmul(
                    pst[:, 0:1], b1[0:1, ts(c, P)], ones32[0:1, 0:1],
                    start=True, stop=True,
                )
                nc.vector.tensor_copy(out=b_encT[:, c : c + 1], in_=pst[:, 0:1])
            for c in range(AC):
                pst = pp_small.tile([P, BL], F32, tag="pssmall")
                nc.tensor.matmul(
                    pst[:, 0:1], w1b[0:1, ts(c, P)], ident16[0:1, 0:1],
                    start=True, stop=True,
                )
                nc.vector.tensor_copy(out=w_attnT[:, c : c + 1], in_=pst[:, 0:1])
            nc.vector.tensor_tensor(
                w_attn_oh,
                w_attnT[:, :, None, None].to_broadcast([P, AC, BL, BL]),
                diag4[:, None, :, :].to_broadcast([P, AC, BL, BL]),
                ALU.mult,
            )
            for c in range(DC):
                pst = pp_small.tile([P, BL], F32, tag="pssmall")
                nc.tensor.matmul(
                    pst, dec16[:, ts(c, P)], ident16[0:BL, 0:BL],
                    start=True, stop=True,
                )
                nc.vector.tensor_copy(out=decT[:, c, :], in_=pst)

            # -------- weight staging: cast-load bf16 -> DRAM -> one 3D
            # mega-transpose ([A, E] contiguous -> [128, EC, A] in one instr) --
            # staging tiles borrow slots from the encnat pool (same shape)
            for w_ap, wT in ((wenc_ap, w_encT), (wdec_ap, w_decT)):
                wst = p_encnat.tile([P, AC, E], BF16, tag="encnat")
                nc.gpsimd.dma_start(wst, w_ap.rearrange("(c p) e -> p c e", p=P))
                wdram = p_dram.tile([A, E], BF16, tag="wdram")
                nc.sync.dma_start(wdram.rearrange("(c p) e -> p c e", p=P), wst)
                nc.sync.dma_start_transpose(wT, wdram)

            # ---------------- dec contribution + fused bias ----------------
            # bias_sb[a_p, ac, b] = (W_dec @ dec.T)[a, b] + b_enc[a]
            for ac_ in range(AC):
                psd = pp_small.tile([P, BL], F32, tag="pssmall")
                for dc_ in range(DC):
                    nc.tensor.matmul(
                        psd,
                        w_decT[:, dc_, ts(ac_, P)],
                        decT[:, dc_, :],
                        start=(dc_ == 0),
                        stop=(dc_ == DC - 1),
                    )
                nc.vector.tensor_scalar_add(
                    bias_sb[:, ac_, :], psd, b_encT[:, ac_ : ac_ + 1]
                )

            # ---------------- per-batch pipeline ----------------
            # all 4 rows' energies accumulate into one [4, S] PSUM tile via the
            # one-hot lhsT columns
            ps_energy = pp_small.tile([BL, S], F32, tag="pssmall")
            encnat_tiles = []
            for b in range(BL):
                encnat = p_encnat.tile([P, SC, E], BF16, tag="encnat")
                encnat_tiles.append(encnat)
                nc.gpsimd.dma_start(
                    encnat, enc_ap[b].rearrange("(c p) e -> p c e", p=P)
                )  # cast fp32->bf16
                edram = p_dram.tile([S, E], BF16, tag="edram")
                nc.sync.dma_start(edram.rearrange("(c p) e -> p c e", p=P), encnat)
                encT = p_encT.tile([P, EC, S], BF16, tag="encT")
                nc.sync.dma_start_transpose(encT, edram)

                pre = p_pre.tile([P, AC, S], BF16, tag="pre")
                for ac_ in range(AC):
                    ps = pp_main.tile([P, S], F32, tag="psmain")
                    for ec_ in range(EC):
                        for nh in range(2):
                            nc.tensor.matmul(
                                ps[:, ts(nh, 512)],
                                w_encT[:, ec_, ts(ac_, P)],
                                encT[:, ec_, ts(nh, 512)],
                                start=(ec_ == 0),
                                stop=(ec_ == EC - 1),
                            )
                    # pre = tanh(enc_contrib.T + dec_contrib.T + b_enc), bf16 out
                    nc.scalar.activation(
                        pre[:, ac_, :], ps, AF.Tanh,
                        bias=bias_sb[:, ac_, b : b + 1], scale=1.0,
                    )
                    for nh in range(2):
                        nc.tensor.matmul(
                            ps_energy[0:BL, ts(nh, 512)],
                            w_attn_oh[:, ac_, b, :],
                            pre[:, ac_, ts(nh, 512)],
                            start=(b == 0 and ac_ == 0),
                            stop=(b == BL - 1 and ac_ == AC - 1),
                        )

            # ---------------- masked softmax over S (all 4 rows) -------------
            # max + exp read the energies straight out of PSUM
            nc.vector.tensor_reduce(stats[:, 0:1], ps_energy, axis=AX.X, op=ALU.max)
            nc.vector.tensor_scalar_mul(stats[:, 1:2], stats[:, 0:1], -1.0)
            nc.scalar.activation(masked, ps_energy, AF.Exp, bias=stats[:, 1:2], scale=1.0)
            # masked = exp * mask; stats[:,2] = rowsum(masked)
            # (tensor_tensor_reduce would fuse these, but the custom DVE op
            # crashes at execute through this runtime path — keep it unfused)
            nc.vector.tensor_tensor(masked, masked, mask_sb, ALU.mult)
            nc.vector.tensor_reduce(stats[:, 2:3], masked, axis=AX.X, op=ALU.add)
            nc.vector.reciprocal(stats[:, 3:4], stats[:, 2:3])
            nc.vector.tensor_scalar_mul(masked, masked, stats[:, 3:4])
            nc.sync.dma_start(alpha_out, masked)
            nc.vector.tensor_copy(out=alpha16, in_=masked)

            # alpha.T: [4, S] -> [s_p, sc, 4] with K=4 identity matmuls
            for sc_ in range(SC):
                pst = pp_small.tile([P, BL], F32, tag="pssmall")
                nc.tensor.matmul(
                    pst, alpha16[:, ts(sc_, P)], ident16[0:BL, 0:BL],
                    start=True, stop=True,
                )
                nc.vector.tensor_copy(out=alphaT[:, sc_, :], in_=pst)
            nc.vector.tensor_tensor(
                alphaT_oh,
                alphaT[:, :, :, None].to_broadcast([P, SC, BL, BL]),
                diag4[:, None, :, :].to_broadcast([P, SC, BL, BL]),
                ALU.mult,
            )

            # ---------------- context = alpha @ enc ----------------
            psc = pp_small.tile([BL, E], F32, tag="pssmall")
            for b in range(BL):
                for sc_ in range(SC):
                    for nh in range(2):
                        nc.tensor.matmul(
                            psc[0:BL, ts(nh, 512)],
                            alphaT_oh[:, sc_, b, :],
                            encnat_tiles[b][:, sc_, ts(nh, 512)],
                            start=(b == 0 and sc_ == 0),
                            stop=(b == BL - 1 and sc_ == SC - 1),
                        )
            nc.vector.tensor_copy(out=ctx_sb, in_=psc)
            nc.sync.dma_start(ctx_out, ctx_sb)

    nc.compile()
    return nc


def _get_compiled():
    global _compiled
    if _compiled is None:
        _compiled = _build_kernel()
    return _compiled


def kernel(dec_state, enc_states, mask, W_enc, b_enc, W_dec, w_attn):
    nc = _get_compiled()

    shared = {
        "W_enc": np.ascontiguousarray(W_enc, dtype=np.float32),
        "b_enc": np.ascontiguousarray(b_enc, dtype=np.float32),
        "W_dec": np.ascontiguousarray(W_dec, dtype=np.float32),
        "w_attn": np.ascontiguousarray(w_attn, dtype=np.float32),
    }
    in_maps = []
    for c in range(NCORES):
        sl = slice(c * BL, (c + 1) * BL)
        in_maps.append(
            {
                "dec_state": np.ascontiguousarray(dec_state[sl], dtype=np.float32),
                "enc_states": np.ascontiguousarray(enc_states[sl], dtype=np.float32),
                "mask": np.ascontiguousarray(mask[sl], dtype=np.float32),
                **shared,
            }
        )

    res = bass_utils.run_bass_kernel_spmd(nc, in_maps, core_ids=list(range(NCORES)))
    kernel.last_results = res
    context = np.concatenate([r["context"] for r in res.results], axis=0)
    alpha = np.concatenate([r["alpha"] for r in res.results], axis=0)
    return (context, alpha)


# revision 28
# speedup vs baseline: 2.9455x; 1.1485x over previous
"""Trainium2 Bass kernel for the tanh-attention module (nn_Attention_1580547969144).

Computation (per batch row b):
    enc_contrib = enc_states @ W_enc.T + b_enc            # [S, A]
    dec_contrib = dec_state @ W_dec.T                     # [A]
    pre = tanh(enc_contrib + dec_contrib)                 # [S, A]
    energy = pre @ w_attn                                 # [S]
    alpha = softmax(energy) * mask; alpha /= sum(alpha)   # [S]
    context = alpha @ enc_states                          # [E]

Sharding: data-parallel over batch, 4 rows per core across 8 NeuronCores.
On-chip strategy (per core):
  - All matmuls run in bf16 with fp32 PSUM accumulation.
  - enc is cast fp32->bf16 during the SWDGE DMA load, round-tripped through a
    DRAM bf16 scratch, and re-loaded with the HWDGE xbar DMA transpose to get
    the [E, S] layout the enc @ W_enc.T contraction needs.  Same staging is
    used once for W_enc.T / W_dec.T.
  - The big [A, S] projection is computed transposed (A on partitions) so that
    the energy reduction over A is a K-partition matmul with w_attn.
  - tanh runs on ScalarE straight out of PSUM with a per-partition fused bias
    (dec_contrib.T + b_enc), writing bf16 to SBUF.
  - Softmax over S for all 4 rows at once on partitions 0..3; the exp*mask and
    its row sum are fused into one tensor_tensor_reduce.
  - context uses the natural-layout bf16 enc tiles kept from the load, with
    alpha.T produced by tiny K=4 identity matmuls.
"""

import sys

if "/opt/trn_rl_repo" not in sys.path:
    sys.path.insert(0, "/opt/trn_rl_repo")

import numpy as np

import concourse.bass as bass  # noqa: F401  (namespace import keeps bass alive)
import concourse.mybir as mybir
import concourse.tile as tile
from concourse import bacc, bass_utils
from concourse.bass import ts
from concourse.masks import make_identity
from concourse.tile_rust import add_dep_helper

B, S, E, A, D = 32, 1024, 1024, 1024, 1024
NCORES = 8
BL = B // NCORES  # 4 batch rows per core
P = 128
SC, EC, AC, DC = S // P, E // P, A // P, D // P
F32 = mybir.dt.float32
BF16 = mybir.dt.bfloat16
AF = mybir.ActivationFunctionType
ALU = mybir.AluOpType
AX = mybir.AxisListType

_compiled = None


def _build_kernel():
    nc = bacc.Bacc(
        "TRN2",
        target_bir_lowering=False,
        debug=False,
        enable_asserts=False,
        num_devices=NCORES,
    )

    dec_ap = nc.dram_tensor("dec_state", [BL, D], F32, kind="ExternalInput").ap()
    enc_ap = nc.dram_tensor("enc_states", [BL, S, E], F32, kind="ExternalInput").ap()
    mask_ap = nc.dram_tensor("mask", [BL, S], F32, kind="ExternalInput").ap()
    wenc_ap = nc.dram_tensor("W_enc", [A, E], F32, kind="ExternalInput").ap()
    benc_ap = nc.dram_tensor("b_enc", [A], F32, kind="ExternalInput").ap()
    wdec_ap = nc.dram_tensor("W_dec", [A, D], F32, kind="ExternalInput").ap()
    wattn_ap = nc.dram_tensor("w_attn", [A], F32, kind="ExternalInput").ap()
    ctx_out = nc.dram_tensor("context", [BL, E], F32, kind="ExternalOutput").ap()
    alpha_out = nc.dram_tensor("alpha", [BL, S], F32, kind="ExternalOutput").ap()

    with tile.TileContext(nc) as tc:
        with (
            tc.tile_pool(name="persist", bufs=1) as persist,
            tc.tile_pool(name="encnat", bufs=BL) as p_encnat,
            tc.tile_pool(name="encT", bufs=2) as p_encT,
            tc.tile_pool(name="pre", bufs=2) as p_pre,
            tc.tile_pool(name="dram", bufs=3, space="DRAM") as p_dram,
            tc.tile_pool(name="ppmain", bufs=2, space="PSUM") as pp_main,
            tc.tile_pool(name="ppsmall", bufs=2, space="PSUM") as pp_small,
        ):
            # ---------------- persistent small tiles ----------------
            ident16 = persist.tile([P, P], BF16, tag="ident16")
            make_identity(nc, ident16)
            ones32 = persist.tile([1, 1], F32, tag="ones32")
            nc.vector.memset(ones32, 1.0)
            # diag4[p, b, m] = (b == m), replicated on every partition
            diag4 = persist.tile([P, BL, BL], BF16, tag="diag4")
            nc.vector.memset(diag4, 0.0)
            for j in range(BL):
                nc.vector.memset(diag4[:, j, j : j + 1], 1.0)

            w_encT = persist.tile([P, EC, A], BF16, tag="w_encT")  # [e_p, ec, a]
            w_decT = persist.tile([P, DC, A], BF16, tag="w_decT")  # [d_p, dc, a]
            b_encT = persist.tile([P, AC], F32, tag="b_encT")  # [a_p, ac]
            w_attnT = persist.tile([P, AC], BF16, tag="w_attnT")  # [a_p, ac]
            # one-hot expansions: [.., b, m] = value * (b == m); column m of the
            # [K, 4] lhsT slice [:, c, b, :] carries the vector only for m == b,
            # so all 4 batch rows can accumulate into one [4, N] PSUM tile
            w_attn_oh = persist.tile([P, AC, BL, BL], BF16, tag="w_attn_oh")
            alphaT_oh = persist.tile([P, SC, BL, BL], BF16, tag="alphaT_oh")
            bias_sb = persist.tile([P, AC, BL], F32, tag="bias_sb")
            dec16 = persist.tile([BL, D], BF16, tag="dec16")
            decT = persist.tile([P, DC, BL], BF16, tag="decT")  # [d_p, dc, b]
            mask_sb = persist.tile([BL, S], F32, tag="mask_sb")
            b1 = persist.tile([1, A], F32, tag="b1")
            w1 = persist.tile([1, A], F32, tag="w1")
            w1b = persist.tile([1, A], BF16, tag="w1b")

            # softmax / output tiles; stats columns: 0=max, 1=-max, 2=sum, 3=1/sum
            stats = persist.tile([BL, 4], F32, tag="stats")
            masked = persist.tile([BL, S], F32, tag="masked")
            alpha16 = persist.tile([BL, S], BF16, tag="alpha16")
            alphaT = persist.tile([P, SC, BL], BF16, tag="alphaT")  # [s_p, sc, b]
            ctx_sb = persist.tile([BL, E], F32, tag="ctx_sb")

            # ---------------- small input loads ----------------
            nc.sync.dma_start(mask_sb, mask_ap)
            nc.sync.dma_start(b1, benc_ap[None, :])
            nc.sync.dma_start(w1, wattn_ap[None, :])
            nc.gpsimd.dma_start(dec16, dec_ap)  # fp32 -> bf16 cast in DMA
            nc.vector.tensor_copy(out=w1b, in_=w1)

            # distribute the 1-D vectors across partitions ([1,128] -> [128,1])
            # via tiny K=1 / K=4 matmuls against an identity column
            for c in range(AC):
                pst = pp_small.tile([P, BL], F32, tag="pssmall")
                nc.tensor.matmul(
                    pst[:, 0:1], b1[0:1, ts(c, P)], ones32[0:1, 0:1],
                    start=True, stop=True,
                )
                nc.vector.tensor_copy(out=b_encT[:, c : c + 1], in_=pst[:, 0:1])
            for c in range(AC):
                pst = pp_small.tile([P, BL], F32, tag="pssmall")
                nc.tensor.matmul(
                    pst[:, 0:1], w1b[0:1, ts(c, P)], ident16[0:1, 0:1],
                    start=True, stop=True,
                )
                nc.vector.tensor_copy(out=w_attnT[:, c : c + 1], in_=pst[:, 0:1])
            nc.vector.tensor_tensor(
                w_attn_oh,
                w_attnT[:, :, None, None].to_broadcast([P, AC, BL, BL]),
                diag4[:, None, :, :].to_broadcast([P, AC, BL, BL]),
                ALU.mult,
            )
            for c in range(DC):
                pst = pp_small.tile([P, BL], F32, tag="pssmall")
                nc.tensor.matmul(
                    pst, dec16[:, ts(c, P)], ident16[0:BL, 0:BL],
                    start=True, stop=True,
                )
                nc.vector.tensor_copy(out=decT[:, c, :], in_=pst)

            # -------- staging helper: cast-load bf16 -> DRAM -> 3D transposes
            # ([R, E] contiguous -> [128, EC, R]), split into e-halves so the
            # first transposed half is available early.  Returns the last
            # transpose instruction (for pipelining deps) and the natural tile.
            H = E // 2

            def stage_transposed(src_ap, dst_T, after=None):
                nat = p_encnat.tile([P, SC, E], BF16, tag="encnat")
                dram = p_dram.tile([S, E], BF16, tag="sdram")
                dram_v = dram.rearrange("(c p) e -> p c e", p=P)
                last = None
                for h in range(2):
                    hs = slice(h * H, (h + 1) * H)
                    cast = nc.gpsimd.dma_start(
                        nat[:, :, hs], src_ap.rearrange("(c p) e -> p c e", p=P)[:, :, hs]
                    )
                    if after is not None:
                        add_dep_helper(cast.ins, after.ins, reason="stage pipelining")
                    nc.sync.dma_start(dram_v[:, :, hs], nat[:, :, hs])
                    last = nc.sync.dma_start_transpose(
                        dst_T[:, 4 * h : 4 * h + 4, :], dram[:, hs]
                    )
                return nat, last

            # ---------------- wave 1: W_enc + enc[0]; wave 2: W_dec ----------
            _, wenc_done = stage_transposed(wenc_ap, w_encT)
            encT0 = p_encT.tile([P, EC, S], BF16, tag="encT")
            encnat0, enc0_done = stage_transposed(enc_ap[0], encT0)
            _, wdec_done = stage_transposed(wdec_ap, w_decT, after=enc0_done)

            # ---------------- dec contribution + fused bias ----------------
            # bias_sb[a_p, ac, b] = (W_dec @ dec.T)[a, b] + b_enc[a]
            for ac_ in range(AC):
                psd = pp_small.tile([P, BL], F32, tag="pssmall")
                for dc_ in range(DC):
                    nc.tensor.matmul(
                        psd,
                        w_decT[:, dc_, ts(ac_, P)],
                        decT[:, dc_, :],
                        start=(dc_ == 0),
                        stop=(dc_ == DC - 1),
                    )
                nc.vector.tensor_scalar_add(
                    bias_sb[:, ac_, :], psd, b_encT[:, ac_ : ac_ + 1]
                )

            # ---------------- per-batch pipeline ----------------
            # all 4 rows' energies accumulate into one [4, S] PSUM tile via the
            # one-hot lhsT columns
            ps_energy = pp_small.tile([BL, S], F32, tag="pssmall")
            encnat_tiles = []
            prev_transp = wdec_done
            for b in range(BL):
                if b == 0:
                    encnat, encT = encnat0, encT0
                else:
                    encT = p_encT.tile([P, EC, S], BF16, tag="encT")
                    # later batches wait for the previous transpose so the SDMA
                    # engines aren't flooded with prefetch reads that starve
                    # the critical staging chain
                    encnat, prev_transp = stage_transposed(
                        enc_ap[b], encT, after=prev_transp
                    )
                encnat_tiles.append(encnat)

                pre = p_pre.tile([P, AC, S], BF16, tag="pre")
                for ac_ in range(AC):
                    ps = pp_main.tile([P, S], F32, tag="psmain")
                    for ec_ in range(EC):
                        for nh in range(2):
                            nc.tensor.matmul(
                                ps[:, ts(nh, 512)],
                                w_encT[:, ec_, ts(ac_, P)],
                                encT[:, ec_, ts(nh, 512)],
                                start=(ec_ == 0),
                                stop=(ec_ == EC - 1),
                            )
                    # pre = tanh(enc_contrib.T + dec_contrib.T + b_enc), bf16 out
                    nc.scalar.activation(
                        pre[:, ac_, :], ps, AF.Tanh,
                        bias=bias_sb[:, ac_, b : b + 1], scale=1.0,
                    )
                    for nh in range(2):
                        nc.tensor.matmul(
                            ps_energy[0:BL, ts(nh, 512)],
                            w_attn_oh[:, ac_, b, :],
                            pre[:, ac_, ts(nh, 512)],
                            start=(b == 0 and ac_ == 0),
                            stop=(b == BL - 1 and ac_ == AC - 1),
                        )

            # ---------------- masked softmax over S (all 4 rows) -------------
            # max + exp read the energies straight out of PSUM
            nc.vector.tensor_reduce(stats[:, 0:1], ps_energy, axis=AX.X, op=ALU.max)
            nc.vector.tensor_scalar_mul(stats[:, 1:2], stats[:, 0:1], -1.0)
            nc.scalar.activation(masked, ps_energy, AF.Exp, bias=stats[:, 1:2], scale=1.0)
            # masked = exp * mask; stats[:,2] = rowsum(masked)
            # (tensor_tensor_reduce would fuse these, but the custom DVE op
            # crashes at execute through this runtime path — keep it unfused)
            nc.vector.tensor_tensor(masked, masked, mask_sb, ALU.mult)
            nc.vector.tensor_reduce(stats[:, 2:3], masked, axis=AX.X, op=ALU.add)
            nc.vector.reciprocal(stats[:, 3:4], stats[:, 2:3])
            nc.vector.tensor_scalar_mul(masked, masked, stats[:, 3:4])
            nc.sync.dma_start(alpha_out, masked)
            nc.vector.tensor_copy(out=alpha16, in_=masked)

            # alpha.T: [4, S] -> [s_p, sc, 4] with K=4 identity matmuls
            for sc_ in range(SC):
                pst = pp_small.tile([P, BL], F32, tag="pssmall")
                nc.tensor.matmul(
                    pst, alpha16[:, ts(sc_, P)], ident16[0:BL, 0:BL],
                    start=True, stop=True,
                )
                nc.vector.tensor_copy(out=alphaT[:, sc_, :], in_=pst)
            nc.vector.tensor_tensor(
                alphaT_oh,
                alphaT[:, :, :, None].to_broadcast([P, SC, BL, BL]),
                diag4[:, None, :, :].to_broadcast([P, SC, BL, BL]),
                ALU.mult,
            )

            # ---------------- context = alpha @ enc ----------------
            psc = pp_small.tile([BL, E], F32, tag="pssmall")
            for b in range(BL):
                for sc_ in range(SC):
                    for nh in range(2):
                        nc.tensor.matmul(
                            psc[0:BL, ts(nh, 512)],
                            alphaT_oh[:, sc_, b, :],
                            encnat_tiles[b][:, sc_, ts(nh, 512)],
                            start=(b == 0 and sc_ == 0),
                            stop=(b == BL - 1 and sc_ == SC - 1),
                        )
            nc.vector.tensor_copy(out=ctx_sb, in_=psc)
            nc.sync.dma_start(ctx_out, ctx_sb)

    nc.compile()
    return nc


def _get_compiled():
    global _compiled
    if _compiled is None:
        _compiled = _build_kernel()
    return _compiled


def kernel(dec_state, enc_states, mask, W_enc, b_enc, W_dec, w_attn):
    nc = _get_compiled()

    shared = {
        "W_enc": np.ascontiguousarray(W_enc, dtype=np.float32),
        "b_enc": np.ascontiguousarray(b_enc, dtype=np.float32),
        "W_dec": np.ascontiguousarray(W_dec, dtype=np.float32),
        "w_attn": np.ascontiguousarray(w_attn, dtype=np.float32),
    }
    in_maps = []
    for c in range(NCORES):
        sl = slice(c * BL, (c + 1) * BL)
        in_maps.append(
            {
                "dec_state": np.ascontiguousarray(dec_state[sl], dtype=np.float32),
                "enc_states": np.ascontiguousarray(enc_states[sl], dtype=np.float32),
                "mask": np.ascontiguousarray(mask[sl], dtype=np.float32),
                **shared,
            }
        )

    res = bass_utils.run_bass_kernel_spmd(nc, in_maps, core_ids=list(range(NCORES)))
    kernel.last_results = res
    context = np.concatenate([r["context"] for r in res.results], axis=0)
    alpha = np.concatenate([r["alpha"] for r in res.results], axis=0)
    return (context, alpha)
